# revision 30
# baseline (speedup 1.0000x reference)
"""Bayesian linear layer (reparameterized) on 8 Trainium2 NeuronCores.

y = x @ (mu + exp(log_sigma) * eps_w).T + (bias_mu + exp(bias_log_sigma) * eps_b)

Shapes: x [8192, 4096] f32, weights [16384, 4096] f32, y [8192, 16384] f32.

Strategy (column-parallel over out_features, 2048 outs per core), hybrid
bf16 + fp8 matmul precision:

  - The PE's bf16 roofline for this problem is ~1.75 ms/core. To beat it,
    8 of the 32 contraction k-tiles run as fp8e4 (e4m3) DoubleRow matmuls
    (2 k-tiles per MM at ~1.13x the cost of one bf16 MM). e4m3 noise on
    one quarter of the contraction puts the output at rel-err ~1.9e-2
    (verified bit-exactly against the reference inputs on host), inside
    the 2e-2 budget; the other 24 k-tiles stay bf16.
  - fp8 needs scale folding (e4m3 has no subnormal headroom at these
    magnitudes): x_fp8 = e4m3(8*x), W_fp8 = e4m3(256*W). So that all
    matmuls accumulate into one PSUM group, the bf16-range W is
    host-prescaled by 2048 = 8*256; eviction applies psum*2^-11 + bias in
    a single DVE scalar_tensor_tensor op.
  - Startup: the old layout idled the PE ~130us while the first W columns
    streamed in. Now a narrow 512-col strip 0 is built k-ordered and swept
    k-outer/m-inner in 4-token-tile blocks (4 psum banks per block,
    alternating halves), so the PE chases the W builder a chunk at a
    time. The remaining 1536 cols build during strip 0's sweep, dribbled
    between blocks so the DVE queue never head-of-line blocks evictions.
  - DMA queues: W-build inputs on sync, x tiles on the scalar engine's
    queue, y stores + bias inputs on gpsimd (SWDGE), so stores and W
    streams never stall the latency-critical x loads.
"""

import os
import sys

sys.path.insert(0, "/opt/trn_rl_repo")
os.environ.setdefault("MYCRO_LOCAL_CACHE", "1")

import numpy as np
import ml_dtypes

N_TOK, IN_DIM, OUT_DIM = 8192, 4096, 16384
N_CORES = 8
OUT_S = OUT_DIM // N_CORES  # 2048
P = 128
CW = 512                    # psum-chunk / W-tile width
KF8_T = 8                   # fp8 k-tiles (of 32); must be even
SX = 8.0                    # x fp8 scale
SW = 256.0                  # W fp8 scale
SB = SX * SW                # bf16-range W host prescale
DESCALE = 1.0 / SB


def build_program(n_tok=N_TOK, in_dim=IN_DIM, out_s=OUT_S, n_cores=N_CORES,
                  kf8_t=KF8_T, xt_bufs=8, out_bufs=5, psum_bufs=8):
    """Build + compile the single-core Bass program (SPMD across cores)."""
    import concourse.bass as bass
    import concourse.mybir as mybir
    import concourse.tile as tile
    from concourse import bacc
    from contextlib import ExitStack

    fp32 = mybir.dt.float32
    bf16 = mybir.dt.bfloat16
    fp16 = mybir.dt.float16
    fp8 = mybir.dt.float8e4
    Exp = mybir.ActivationFunctionType.Exp
    add = mybir.AluOpType.add
    mult = mybir.AluOpType.mult
    DR = mybir.MatmulPerfMode.DoubleRow

    KT = in_dim // P
    MT = n_tok // P
    NCH = out_s // CW
    assert in_dim % P == 0 and n_tok % P == 0 and out_s % CW == 0
    assert kf8_t % 2 == 0 and 0 <= kf8_t < KT
    KBF_T = KT - kf8_t          # bf16 k-tiles
    NPAIR = kf8_t // 2          # fp8 DoubleRow pairs
    KF8 = kf8_t * P             # fp8 contraction rows

    nc = bacc.Bacc("TRN2", target_bir_lowering=False, debug=False,
                   num_devices=n_cores, enable_asserts=False)

    # x pre-tiled on host: xB[m, ki, kb, t] = x[m*128 + t, KF8 + kb*128 + ki]
    xB = nc.dram_tensor("xB", [MT, P, KBF_T, P], bf16, kind="ExternalInput")
    if kf8_t:
        # xB8[m, ki, kf, t] = e4m3(8 * x[m*128 + t, kf*128 + ki])
        xB8 = nc.dram_tensor("xB8", [MT, P, kf8_t, P], fp8,
                             kind="ExternalInput")
        # packed (ls | mu*SW | eps*SW) per fp8 k-tile and chunk
        wle8 = nc.dram_tensor("wle8", [kf8_t, NCH, P, 3, CW], fp16,
                              kind="ExternalInput")
    # fp16 inputs: ls ~ -5 and bf16's 8-bit mantissa on ls would be a ~1%
    # multiplicative error after exp; fp16 keeps the bf16-range W at
    # f32-input accuracy at half the DMA traffic. mu/eps carry the 2048x
    # (bf16 range) / 256x (fp8 range) scale from the host. The three
    # tensors are packed (ls | mu | eps) per (k-tile, chunk) so one W
    # chunk costs ONE dma_start on the issuing engine, not three.
    wle = nc.dram_tensor("wle", [KBF_T, NCH, P, 3, CW], fp16,
                         kind="ExternalInput")
    bmu = nc.dram_tensor("bmu", [out_s], fp32, kind="ExternalInput")
    bls = nc.dram_tensor("bls", [out_s], fp32, kind="ExternalInput")
    beps = nc.dram_tensor("beps", [out_s], fp32, kind="ExternalInput")
    y = nc.dram_tensor("y", [n_tok, out_s], fp32, kind="ExternalOutput")

    with tile.TileContext(nc) as tc, ExitStack() as ctx:
        wt_pool = ctx.enter_context(tc.tile_pool(name="wt", bufs=1))
        const_pool = ctx.enter_context(tc.tile_pool(name="const", bufs=1))
        scratch = ctx.enter_context(tc.tile_pool(name="scratch", bufs=2))
        xt_pool = ctx.enter_context(tc.tile_pool(name="xt", bufs=xt_bufs))
        out_pool = ctx.enter_context(tc.tile_pool(name="out", bufs=out_bufs))
        psum_pool = ctx.enter_context(
            tc.tile_pool(name="psum", bufs=psum_bufs, space="PSUM"))

        def fused_w(dst_ap, packed_src):
            # dst = mu + exp(ls) * eps from a packed [P, 3, CW] input:
            # one dma_start per W chunk (a dma_start costs ~0.7us of the
            # issuing engine, so 3-per-chunk made the sync engine itself a
            # serial bottleneck during the strip-0 chase). gpsimd tensor
            # ops measure 2x slower than DVE and drag ACT/DVE via port
            # contention - keep the elementwise work on DVE.
            t = scratch.tile([P, 3, CW], fp16, tag="wle", name="wle_t")
            x_ = scratch.tile([P, CW], fp32, tag="exp", name="exp_t")
            nc.sync.dma_start(out=t[:], in_=packed_src)
            nc.scalar.activation(x_[:], t[:, 0, :], Exp)
            nc.vector.tensor_mul(x_[:], x_[:], t[:, 2, :])
            nc.vector.tensor_tensor(dst_ap, x_[:], t[:, 1, :], add)

        # bias_rep[p, o] = bmu[o] + exp(bls[o]) * beps[o], natural scale
        bias_rep = const_pool.tile([P, out_s], bf16, tag="bias_rep",
                                   name="bias_rep")

        def bias_chunk(c):
            sl = slice(c * CW, (c + 1) * CW)
            l = scratch.tile([P, CW], fp32, tag="bls", name="bls_t", bufs=1)
            e = scratch.tile([P, CW], fp32, tag="beps", name="beps_t", bufs=1)
            m_ = scratch.tile([P, CW], fp32, tag="bmu", name="bmu_t", bufs=1)
            x_ = scratch.tile([P, CW], fp32, tag="bexp", name="bexp_t",
                              bufs=1)
            nc.gpsimd.dma_start(out=l[:], in_=bls.ap()[sl].partition_broadcast(P))
            nc.gpsimd.dma_start(out=e[:], in_=beps.ap()[sl].partition_broadcast(P))
            nc.gpsimd.dma_start(out=m_[:], in_=bmu.ap()[sl].partition_broadcast(P))
            nc.scalar.activation(x_[:], l[:], Exp)
            nc.vector.tensor_mul(x_[:], x_[:], e[:])
            nc.vector.tensor_tensor(bias_rep[:, sl], x_[:], m_[:], add)

        # ---- W tiles (SBUF-resident for the whole kernel) ----
        wt = {}    # (kb, c) -> [P, CW] bf16
        w8 = {}    # (pj, c) -> [P, 2, CW] fp8

        def build_wt(kb, c):
            t = wt_pool.tile([P, CW], bf16, tag=f"wt{kb}_{c}",
                             name=f"wt{kb}_{c}")
            wt[(kb, c)] = t
            fused_w(t[:], wle.ap()[kb, c])

        def build_w8(pj, c):
            t = wt_pool.tile([P, 2, CW], fp8, tag=f"w8{pj}_{c}",
                             name=f"w8{pj}_{c}")
            w8[(pj, c)] = t
            for i in range(2):
                fused_w(t[:, i, :], wle8.ap()[2 * pj + i, c])

        # x on the scalar ring, W on the sync ring, stores on gpsimd.
        # (Measured dead ends: x split across rings, x or W via the
        # gpsimd SWDGE ring, W-build compute on gpsimd - every variant
        # lost 50-400us to FIFO head-of-line coupling or slow gpsimd ALUs.)
        def load_x(m, startup=False):
            xt = xt_pool.tile([P, KBF_T, P], bf16, tag="xt", name="xt")
            nc.scalar.dma_start(out=xt[:], in_=xB.ap()[m])
            xt8 = None
            if kf8_t:
                xt8 = xt_pool.tile([P, kf8_t, P], fp8, tag="xt8", name="xt8")
                nc.scalar.dma_start(out=xt8[:], in_=xB8.ap()[m])
            return xt, xt8

        def k_sweep(psum_of_m, xts, c):
            """Emit the full contraction for psum chunk c over the given
            token tiles. xts: list of (m, xt, xt8). For each k-entity the
            inner loop runs over token tiles (so strip-0 blocks chase the
            W builder chunk by chunk). bf16 first: its first chunk is the
            cheapest build, so the PE starts earliest."""
            for kb in range(KBF_T):
                for m, xt, xt8 in xts:
                    nc.tensor.matmul(
                        psum_of_m[m][:], xt[:, kb, :], wt[(kb, c)][:],
                        start=(kb == 0), stop=(kb == KBF_T - 1 and
                                               NPAIR == 0))
            for pj in range(NPAIR):
                for m, xt, xt8 in xts:
                    nc.tensor.matmul(
                        psum_of_m[m][:], xt8[:, 2 * pj:2 * pj + 2, :],
                        w8[(pj, c)][:], start=(KBF_T == 0 and pj == 0),
                        stop=(pj == NPAIR - 1),
                        perf_mode=DR)

        def evict(ps, m, c):
            ot = out_pool.tile([P, CW], fp32, tag="ot", name="ot")
            # y = psum * 2^-11 + bias in one DVE op; store via SWDGE so the
            # write never head-of-line-blocks a load queue.
            nc.vector.scalar_tensor_tensor(
                ot[:], ps[:], DESCALE, bias_rep[:, c * CW:(c + 1) * CW],
                mult, add)
            nc.gpsimd.dma_start(
                out=y.ap()[m * P:(m + 1) * P, c * CW:(c + 1) * CW],
                in_=ot[:])

        # ---- emission ----
        x_tiles = {}
        blocks = [list(range(m0, min(m0 + 4, MT)))
                  for m0 in range(0, MT, 4)]

        for m in blocks[0]:
            x_tiles[m] = load_x(m, startup=True)

        # Warm-up: throwaway matmuls keep the PE dense through the first
        # W-build window so the HAM clock gate opens to 8/8 and stays.
        if KBF_T >= 2:
            xt0 = x_tiles[blocks[0][0]][0]
            warm_ps = psum_pool.tile([P, CW], fp32, tag="ps", name="warm_ps")
            for _ in range(50):
                nc.tensor.matmul(warm_ps[:, :P], xt0[:, 0, :], xt0[:, 1, :],
                                 start=True, stop=True)

        def strip_jobs(c):
            jobs = [(lambda kb=kb, c=c: build_wt(kb, c))
                    for kb in range(KBF_T)]
            jobs += [(lambda pj=pj, c=c: build_w8(pj, c))
                     for pj in range(NPAIR)]
            jobs.append(lambda c=c: bias_chunk(c))
            return jobs

        # Strip-0 builds first, k-ordered, so phase B's blocks chase the
        # builder chunk by chunk from the first matmul on.
        for job in strip_jobs(0):
            job()

        # Remaining strips' builds, dribbled between strip-0 blocks.
        pending = [j for c in range(1, NCH) for j in strip_jobs(c)]
        n_pending = len(pending)
        pending = iter(pending)
        per_blk = -(-n_pending // max(len(blocks) - 1, 1))

        # Phase B: strip 0 (chunk 0), k-outer / m-inner per block.
        for bi, blk in enumerate(blocks):
            if bi + 1 < len(blocks):
                for m in blocks[bi + 1]:
                    x_tiles[m] = load_x(m, startup=(bi == 0))
            psums = {m: psum_pool.tile([P, CW], fp32, tag="ps",
                                       name=f"ps{m}") for m in blk}
            k_sweep(psums, [(m,) + x_tiles[m] for m in blk], 0)
            for m in blk:
                evict(psums[m], m, 0)
                del x_tiles[m]
            if bi >= 1:
                for _ in range(per_blk):
                    job = next(pending, None)
                    if job is not None:
                        job()
        for job in pending:
            job()

        # Phase C: strips 1..NCH-1, m-outer with per-m chunk fan-out.
        if NCH > 1:
            for m in range(MT):
                xt, xt8 = load_x(m)
                psc = {c: psum_pool.tile([P, CW], fp32, tag="ps",
                                         name=f"pc{m}_{c}")
                       for c in range(1, NCH)}
                for kb in range(KBF_T):
                    lhsT = xt[:, kb, :]
                    for c in range(1, NCH):
                        nc.tensor.matmul(psc[c][:], lhsT, wt[(kb, c)][:],
                                         start=(kb == 0),
                                         stop=(kb == KBF_T - 1 and
                                               NPAIR == 0))
                for pj in range(NPAIR):
                    lhsT = xt8[:, 2 * pj:2 * pj + 2, :]
                    for c in range(1, NCH):
                        nc.tensor.matmul(psc[c][:], lhsT, w8[(pj, c)][:],
                                         start=(KBF_T == 0 and pj == 0),
                                         stop=(pj == NPAIR - 1),
                                         perf_mode=DR)
                for c in range(1, NCH):
                    evict(psc[c], m, c)

    nc.compile()
    return nc


_PROGRAM_CACHE = {}


def _get_program():
    key = (N_TOK, IN_DIM, OUT_S, KF8_T)
    if key not in _PROGRAM_CACHE:
        _PROGRAM_CACHE[key] = build_program()
    return _PROGRAM_CACHE[key]


def _pack3(lsA, muA, epsA, kt, out_s):
    """Pack (ls | mu | eps), each [kt*128, out_s], to [kt, NCH, 128, 3, CW]."""
    nch = out_s // CW

    def r(a):
        return np.asarray(a, dtype=np.float32).reshape(kt, P, nch, CW)

    a = np.stack([r(lsA), r(muA), r(epsA)], axis=3)  # [kt, P, nch, 3, CW]
    return np.ascontiguousarray(a.transpose(0, 2, 1, 3, 4)).astype(np.float16)


def make_in_maps(x, weight_mu, weight_log_sigma, bias_mu, bias_log_sigma,
                 eps_w, eps_b, kf8_t=KF8_T):
    x = np.asarray(x, dtype=np.float32)
    weight_mu = np.asarray(weight_mu, dtype=np.float32)
    weight_log_sigma = np.asarray(weight_log_sigma, dtype=np.float32)
    bias_mu = np.asarray(bias_mu, dtype=np.float32)
    bias_log_sigma = np.asarray(bias_log_sigma, dtype=np.float32)
    eps_w = np.asarray(eps_w, dtype=np.float32)
    eps_b = np.asarray(eps_b, dtype=np.float32)

    MT, KT = N_TOK // P, IN_DIM // P
    KF8 = kf8_t * P
    xr = x.reshape(MT, P, KT, P)  # [m, t, ko, ki]
    xB = np.ascontiguousarray(
        xr[:, :, kf8_t:, :].transpose(0, 3, 2, 1)).astype(ml_dtypes.bfloat16)
    xB8 = np.ascontiguousarray(
        (xr[:, :, :kf8_t, :] * SX).transpose(0, 3, 2, 1)).astype(
            ml_dtypes.float8_e4m3)
    in_maps = []
    for c in range(N_CORES):
        sl = slice(c * OUT_S, (c + 1) * OUT_S)
        im = {
            "xB": xB,
            "wle": _pack3(weight_log_sigma[sl, KF8:].T,
                          (weight_mu[sl, KF8:] * SB).T,
                          (eps_w[sl, KF8:] * SB).T,
                          (IN_DIM - KF8) // P, OUT_S),
            "bmu": np.ascontiguousarray(bias_mu[sl]),
            "bls": np.ascontiguousarray(bias_log_sigma[sl]),
            "beps": np.ascontiguousarray(eps_b[sl]),
        }
        if kf8_t:
            im["xB8"] = xB8
            im["wle8"] = _pack3(weight_log_sigma[sl, :KF8].T,
                                (weight_mu[sl, :KF8] * SW).T,
                                (eps_w[sl, :KF8] * SW).T,
                                kf8_t, OUT_S)
        in_maps.append(im)
    return in_maps


def run(in_maps, trace=False, **kwargs):
    import time
    from concourse.bass_utils import run_bass_kernel_spmd
    nc = _get_program()
    for attempt in range(3):
        try:
            res = run_bass_kernel_spmd(nc, in_maps, list(range(N_CORES)),
                                       trace=trace, **kwargs)
            break
        except Exception:  # transient NRT_EXEC_UNIT_UNRECOVERABLE
            if attempt == 2:
                raise
            time.sleep(15)
    out = np.concatenate([res.results[c]["y"] for c in range(N_CORES)], axis=1)
    return out, res


def kernel(x, weight_mu, weight_log_sigma, bias_mu, bias_log_sigma,
           eps_w, eps_b):
    in_maps = make_in_maps(x, weight_mu, weight_log_sigma, bias_mu,
                           bias_log_sigma, eps_w, eps_b)
    out, _ = run(in_maps, trace=False)
    return out


# revision 31
# speedup vs baseline: 1.0316x; 1.0316x over previous
"""Bayesian linear layer (reparameterized) on 8 Trainium2 NeuronCores.

y = x @ (mu + exp(log_sigma) * eps_w).T + (bias_mu + exp(bias_log_sigma) * eps_b)

Shapes: x [8192, 4096] f32, weights [16384, 4096] f32, y [8192, 16384] f32.

Strategy (column-parallel over out_features, 2048 outs per core), hybrid
bf16 + fp8 matmul precision:

  - The PE's bf16 roofline for this problem is ~1.75 ms/core. To beat it,
    8 of the 32 contraction k-tiles run as fp8e4 (e4m3) DoubleRow matmuls
    (2 k-tiles per MM at ~1.13x the cost of one bf16 MM). e4m3 noise on
    one quarter of the contraction puts the output at rel-err ~1.9e-2
    (verified bit-exactly against the reference inputs on host), inside
    the 2e-2 budget; the other 24 k-tiles stay bf16.
  - fp8 needs scale folding (e4m3 has no subnormal headroom at these
    magnitudes): x_fp8 = e4m3(8*x), W_fp8 = e4m3(256*W). So that all
    matmuls accumulate into one PSUM group, the bf16-range W is
    host-prescaled by 2048 = 8*256; eviction applies psum*2^-11 + bias in
    a single DVE scalar_tensor_tensor op.
  - Startup: the old layout idled the PE ~130us while the first W columns
    streamed in. Now a narrow 512-col strip 0 is built k-ordered and swept
    k-outer/m-inner in 4-token-tile blocks (4 psum banks per block,
    alternating halves), so the PE chases the W builder a chunk at a
    time. The remaining 1536 cols build during strip 0's sweep, dribbled
    between blocks so the DVE queue never head-of-line blocks evictions.
  - DMA queues: W-build inputs on sync, x tiles on the scalar engine's
    queue, y stores + bias inputs on gpsimd (SWDGE), so stores and W
    streams never stall the latency-critical x loads.
"""

import os
import sys

sys.path.insert(0, "/opt/trn_rl_repo")
os.environ.setdefault("MYCRO_LOCAL_CACHE", "1")

import numpy as np
import ml_dtypes

N_TOK, IN_DIM, OUT_DIM = 8192, 4096, 16384
N_CORES = 8
OUT_S = OUT_DIM // N_CORES  # 2048
P = 128
CW = 512                    # psum-chunk / W-tile width
KF8_T = 8                   # fp8 k-tiles (of 32); must be even
SX = 8.0                    # x fp8 scale
SW = 256.0                  # W fp8 scale
SB = SX * SW                # bf16-range W host prescale
DESCALE = 1.0 / SB


def build_program(n_tok=N_TOK, in_dim=IN_DIM, out_s=OUT_S, n_cores=N_CORES,
                  kf8_t=KF8_T, xt_bufs=8, out_bufs=5, psum_bufs=8):
    """Build + compile the single-core Bass program (SPMD across cores)."""
    import concourse.bass as bass
    import concourse.mybir as mybir
    import concourse.tile as tile
    from concourse import bacc
    from contextlib import ExitStack

    fp32 = mybir.dt.float32
    bf16 = mybir.dt.bfloat16
    fp16 = mybir.dt.float16
    fp8 = mybir.dt.float8e4
    Exp = mybir.ActivationFunctionType.Exp
    add = mybir.AluOpType.add
    mult = mybir.AluOpType.mult
    DR = mybir.MatmulPerfMode.DoubleRow

    KT = in_dim // P
    MT = n_tok // P
    NCH = out_s // CW
    assert in_dim % P == 0 and n_tok % P == 0 and out_s % CW == 0
    assert kf8_t % 2 == 0 and 0 <= kf8_t < KT
    KBF_T = KT - kf8_t          # bf16 k-tiles
    NPAIR = kf8_t // 2          # fp8 DoubleRow pairs
    KF8 = kf8_t * P             # fp8 contraction rows

    nc = bacc.Bacc("TRN2", target_bir_lowering=False, debug=False,
                   num_devices=n_cores, enable_asserts=False)

    # x pre-tiled on host: xB[m, ki, kb, t] = x[m*128 + t, KF8 + kb*128 + ki]
    xB = nc.dram_tensor("xB", [MT, P, KBF_T, P], bf16, kind="ExternalInput")
    if kf8_t:
        # xB8[m, ki, kf, t] = e4m3(8 * x[m*128 + t, kf*128 + ki])
        xB8 = nc.dram_tensor("xB8", [MT, P, kf8_t, P], fp8,
                             kind="ExternalInput")
        # packed (ls | mu*SW | eps*SW) per fp8 k-tile and chunk
        wle8 = nc.dram_tensor("wle8", [kf8_t, NCH, P, 3, CW], fp16,
                              kind="ExternalInput")
    # fp16 inputs: ls ~ -5 and bf16's 8-bit mantissa on ls would be a ~1%
    # multiplicative error after exp; fp16 keeps the bf16-range W at
    # f32-input accuracy at half the DMA traffic. mu/eps carry the 2048x
    # (bf16 range) / 256x (fp8 range) scale from the host. The three
    # tensors are packed (ls | mu | eps) per (k-tile, chunk) so one W
    # chunk costs ONE dma_start on the issuing engine, not three.
    wle = nc.dram_tensor("wle", [KBF_T, NCH, P, 3, CW], fp16,
                         kind="ExternalInput")
    bmu = nc.dram_tensor("bmu", [out_s], fp32, kind="ExternalInput")
    bls = nc.dram_tensor("bls", [out_s], fp32, kind="ExternalInput")
    beps = nc.dram_tensor("beps", [out_s], fp32, kind="ExternalInput")
    y = nc.dram_tensor("y", [n_tok, out_s], fp32, kind="ExternalOutput")

    with tile.TileContext(nc) as tc, ExitStack() as ctx:
        wt_pool = ctx.enter_context(tc.tile_pool(name="wt", bufs=1))
        const_pool = ctx.enter_context(tc.tile_pool(name="const", bufs=1))
        scratch = ctx.enter_context(tc.tile_pool(name="scratch", bufs=2))
        xt_pool = ctx.enter_context(tc.tile_pool(name="xt", bufs=xt_bufs))
        out_pool = ctx.enter_context(tc.tile_pool(name="out", bufs=out_bufs))
        psum_pool = ctx.enter_context(
            tc.tile_pool(name="psum", bufs=psum_bufs, space="PSUM"))

        def fused_w(dst_ap, packed_src):
            # dst = mu + exp(ls) * eps from a packed [P, 3, CW] input:
            # one dma_start per W chunk (a dma_start costs ~0.7us of the
            # issuing engine, so 3-per-chunk made the sync engine itself a
            # serial bottleneck during the strip-0 chase). gpsimd tensor
            # ops measure 2x slower than DVE and drag ACT/DVE via port
            # contention - keep the elementwise work on DVE.
            # ls in its own DMA so exp starts after ~1us, not after the
            # whole 3KB/partition chunk lands; mu+eps share the second.
            l = scratch.tile([P, CW], fp16, tag="ls", name="ls_t")
            me = scratch.tile([P, 2, CW], fp16, tag="me", name="me_t")
            x_ = scratch.tile([P, CW], fp32, tag="exp", name="exp_t")
            nc.sync.dma_start(out=l[:], in_=packed_src[:, 0, :])
            nc.sync.dma_start(out=me[:], in_=packed_src[:, 1:3, :])
            nc.scalar.activation(x_[:], l[:], Exp)
            nc.vector.tensor_mul(x_[:], x_[:], me[:, 1, :])
            nc.vector.tensor_tensor(dst_ap, x_[:], me[:, 0, :], add)

        # bias_rep[p, o] = bmu[o] + exp(bls[o]) * beps[o], natural scale
        bias_rep = const_pool.tile([P, out_s], bf16, tag="bias_rep",
                                   name="bias_rep")

        def bias_chunk(c):
            sl = slice(c * CW, (c + 1) * CW)
            l = scratch.tile([P, CW], fp32, tag="bls", name="bls_t", bufs=1)
            e = scratch.tile([P, CW], fp32, tag="beps", name="beps_t", bufs=1)
            m_ = scratch.tile([P, CW], fp32, tag="bmu", name="bmu_t", bufs=1)
            x_ = scratch.tile([P, CW], fp32, tag="bexp", name="bexp_t",
                              bufs=1)
            nc.gpsimd.dma_start(out=l[:], in_=bls.ap()[sl].partition_broadcast(P))
            nc.gpsimd.dma_start(out=e[:], in_=beps.ap()[sl].partition_broadcast(P))
            nc.gpsimd.dma_start(out=m_[:], in_=bmu.ap()[sl].partition_broadcast(P))
            nc.scalar.activation(x_[:], l[:], Exp)
            nc.vector.tensor_mul(x_[:], x_[:], e[:])
            nc.vector.tensor_tensor(bias_rep[:, sl], x_[:], m_[:], add)

        # ---- W tiles (SBUF-resident for the whole kernel) ----
        wt = {}    # (kb, c) -> [P, CW] bf16
        w8 = {}    # (pj, c) -> [P, 2, CW] fp8

        def build_wt(kb, c):
            t = wt_pool.tile([P, CW], bf16, tag=f"wt{kb}_{c}",
                             name=f"wt{kb}_{c}")
            wt[(kb, c)] = t
            fused_w(t[:], wle.ap()[kb, c])

        def build_w8(pj, c):
            t = wt_pool.tile([P, 2, CW], fp8, tag=f"w8{pj}_{c}",
                             name=f"w8{pj}_{c}")
            w8[(pj, c)] = t
            for i in range(2):
                fused_w(t[:, i, :], wle8.ap()[2 * pj + i, c])

        # x on the scalar ring, W on the sync ring, stores on gpsimd.
        # (Measured dead ends: x split across rings, x or W via the
        # gpsimd SWDGE ring, W-build compute on gpsimd - every variant
        # lost 50-400us to FIFO head-of-line coupling or slow gpsimd ALUs.)
        def load_x(m, startup=False):
            xt = xt_pool.tile([P, KBF_T, P], bf16, tag="xt", name="xt")
            nc.scalar.dma_start(out=xt[:], in_=xB.ap()[m])
            xt8 = None
            if kf8_t:
                xt8 = xt_pool.tile([P, kf8_t, P], fp8, tag="xt8", name="xt8")
                nc.scalar.dma_start(out=xt8[:], in_=xB8.ap()[m])
            return xt, xt8

        def k_sweep(psum_of_m, xts, c):
            """Emit the full contraction for psum chunk c over the given
            token tiles. xts: list of (m, xt, xt8). For each k-entity the
            inner loop runs over token tiles (so strip-0 blocks chase the
            W builder chunk by chunk). bf16 first: its first chunk is the
            cheapest build, so the PE starts earliest."""
            for kb in range(KBF_T):
                for m, xt, xt8 in xts:
                    nc.tensor.matmul(
                        psum_of_m[m][:], xt[:, kb, :], wt[(kb, c)][:],
                        start=(kb == 0), stop=(kb == KBF_T - 1 and
                                               NPAIR == 0))
            for pj in range(NPAIR):
                for m, xt, xt8 in xts:
                    nc.tensor.matmul(
                        psum_of_m[m][:], xt8[:, 2 * pj:2 * pj + 2, :],
                        w8[(pj, c)][:], start=(KBF_T == 0 and pj == 0),
                        stop=(pj == NPAIR - 1),
                        perf_mode=DR)

        def evict(ps, m, c):
            ot = out_pool.tile([P, CW], fp32, tag="ot", name="ot")
            # y = psum * 2^-11 + bias in one DVE op; store via SWDGE so the
            # write never head-of-line-blocks a load queue.
            nc.vector.scalar_tensor_tensor(
                ot[:], ps[:], DESCALE, bias_rep[:, c * CW:(c + 1) * CW],
                mult, add)
            nc.gpsimd.dma_start(
                out=y.ap()[m * P:(m + 1) * P, c * CW:(c + 1) * CW],
                in_=ot[:])

        # ---- emission ----
        x_tiles = {}
        blocks = [list(range(m0, min(m0 + 4, MT)))
                  for m0 in range(0, MT, 4)]

        for m in blocks[0]:
            x_tiles[m] = load_x(m, startup=True)

        # Warm-up: throwaway matmuls keep the PE dense through the first
        # W-build window so the HAM clock gate opens to 8/8 and stays.
        if KBF_T >= 2:
            xt0 = x_tiles[blocks[0][0]][0]
            warm_ps = psum_pool.tile([P, CW], fp32, tag="ps", name="warm_ps")
            for _ in range(50):
                nc.tensor.matmul(warm_ps[:, :P], xt0[:, 0, :], xt0[:, 1, :],
                                 start=True, stop=True)

        def strip_jobs(c):
            jobs = [(lambda kb=kb, c=c: build_wt(kb, c))
                    for kb in range(KBF_T)]
            jobs += [(lambda pj=pj, c=c: build_w8(pj, c))
                     for pj in range(NPAIR)]
            jobs.append(lambda c=c: bias_chunk(c))
            return jobs

        # Strip-0 builds first, k-ordered, so phase B's blocks chase the
        # builder chunk by chunk from the first matmul on.
        for job in strip_jobs(0):
            job()

        # Remaining strips' builds, dribbled between strip-0 blocks.
        pending = [j for c in range(1, NCH) for j in strip_jobs(c)]
        n_pending = len(pending)
        pending = iter(pending)
        per_blk = -(-n_pending // max(len(blocks) - 1, 1))

        # Phase B: strip 0 (chunk 0), k-outer / m-inner per block.
        for bi, blk in enumerate(blocks):
            if bi + 1 < len(blocks):
                for m in blocks[bi + 1]:
                    x_tiles[m] = load_x(m, startup=(bi == 0))
            psums = {m: psum_pool.tile([P, CW], fp32, tag="ps",
                                       name=f"ps{m}") for m in blk}
            k_sweep(psums, [(m,) + x_tiles[m] for m in blk], 0)
            for m in blk:
                evict(psums[m], m, 0)
                del x_tiles[m]
            if bi >= 1:
                for _ in range(per_blk):
                    job = next(pending, None)
                    if job is not None:
                        job()
        for job in pending:
            job()

        # Phase C: strips 1..NCH-1, m-outer with per-m chunk fan-out.
        if NCH > 1:
            for m in range(MT):
                xt, xt8 = load_x(m)
                psc = {c: psum_pool.tile([P, CW], fp32, tag="ps",
                                         name=f"pc{m}_{c}")
                       for c in range(1, NCH)}
                for kb in range(KBF_T):
                    lhsT = xt[:, kb, :]
                    for c in range(1, NCH):
                        nc.tensor.matmul(psc[c][:], lhsT, wt[(kb, c)][:],
                                         start=(kb == 0),
                                         stop=(kb == KBF_T - 1 and
                                               NPAIR == 0))
                for pj in range(NPAIR):
                    lhsT = xt8[:, 2 * pj:2 * pj + 2, :]
                    for c in range(1, NCH):
                        nc.tensor.matmul(psc[c][:], lhsT, w8[(pj, c)][:],
                                         start=(KBF_T == 0 and pj == 0),
                                         stop=(pj == NPAIR - 1),
                                         perf_mode=DR)
                for c in range(1, NCH):
                    evict(psc[c], m, c)

    nc.compile()
    return nc


_PROGRAM_CACHE = {}


def _get_program():
    key = (N_TOK, IN_DIM, OUT_S, KF8_T)
    if key not in _PROGRAM_CACHE:
        _PROGRAM_CACHE[key] = build_program()
    return _PROGRAM_CACHE[key]


def _pack3(lsA, muA, epsA, kt, out_s):
    """Pack (ls | mu | eps), each [kt*128, out_s], to [kt, NCH, 128, 3, CW]."""
    nch = out_s // CW

    def r(a):
        return np.asarray(a, dtype=np.float32).reshape(kt, P, nch, CW)

    a = np.stack([r(lsA), r(muA), r(epsA)], axis=3)  # [kt, P, nch, 3, CW]
    return np.ascontiguousarray(a.transpose(0, 2, 1, 3, 4)).astype(np.float16)


def make_in_maps(x, weight_mu, weight_log_sigma, bias_mu, bias_log_sigma,
                 eps_w, eps_b, kf8_t=KF8_T):
    x = np.asarray(x, dtype=np.float32)
    weight_mu = np.asarray(weight_mu, dtype=np.float32)
    weight_log_sigma = np.asarray(weight_log_sigma, dtype=np.float32)
    bias_mu = np.asarray(bias_mu, dtype=np.float32)
    bias_log_sigma = np.asarray(bias_log_sigma, dtype=np.float32)
    eps_w = np.asarray(eps_w, dtype=np.float32)
    eps_b = np.asarray(eps_b, dtype=np.float32)

    MT, KT = N_TOK // P, IN_DIM // P
    KF8 = kf8_t * P
    xr = x.reshape(MT, P, KT, P)  # [m, t, ko, ki]
    xB = np.ascontiguousarray(
        xr[:, :, kf8_t:, :].transpose(0, 3, 2, 1)).astype(ml_dtypes.bfloat16)
    xB8 = np.ascontiguousarray(
        (xr[:, :, :kf8_t, :] * SX).transpose(0, 3, 2, 1)).astype(
            ml_dtypes.float8_e4m3)
    in_maps = []
    for c in range(N_CORES):
        sl = slice(c * OUT_S, (c + 1) * OUT_S)
        im = {
            "xB": xB,
            "wle": _pack3(weight_log_sigma[sl, KF8:].T,
                          (weight_mu[sl, KF8:] * SB).T,
                          (eps_w[sl, KF8:] * SB).T,
                          (IN_DIM - KF8) // P, OUT_S),
            "bmu": np.ascontiguousarray(bias_mu[sl]),
            "bls": np.ascontiguousarray(bias_log_sigma[sl]),
            "beps": np.ascontiguousarray(eps_b[sl]),
        }
        if kf8_t:
            im["xB8"] = xB8
            im["wle8"] = _pack3(weight_log_sigma[sl, :KF8].T,
                                (weight_mu[sl, :KF8] * SW).T,
                                (eps_w[sl, :KF8] * SW).T,
                                kf8_t, OUT_S)
        in_maps.append(im)
    return in_maps


def run(in_maps, trace=False, **kwargs):
    import time
    from concourse.bass_utils import run_bass_kernel_spmd
    nc = _get_program()
    for attempt in range(3):
        try:
            res = run_bass_kernel_spmd(nc, in_maps, list(range(N_CORES)),
                                       trace=trace, **kwargs)
            break
        except Exception:  # transient NRT_EXEC_UNIT_UNRECOVERABLE
            if attempt == 2:
                raise
            time.sleep(15)
    out = np.concatenate([res.results[c]["y"] for c in range(N_CORES)], axis=1)
    return out, res


def kernel(x, weight_mu, weight_log_sigma, bias_mu, bias_log_sigma,
           eps_w, eps_b):
    in_maps = make_in_maps(x, weight_mu, weight_log_sigma, bias_mu,
                           bias_log_sigma, eps_w, eps_b)
    out, _ = run(in_maps, trace=False)
    return out
